# revision 57
# baseline (speedup 1.0000x reference)
"""Trainium2 Bass kernel for a dense transformer block (nn_Block_30262339567972).

Full inputs in, full outputs out. Head-sharded across 8 NeuronCores with one
ReduceScatter per head-pair group:

  core c = 2*b + j  (batch b, half j) owns
    - attention: GLOBAL heads [8j, 8j+8) over the full 2048-token sequence
    - MLP/residual/output: tokens [1024j, 1024(j+1)) of batch b

Each core computes LN1 for the full sequence once, projects Q/K/V only for
its 8 heads (halving K/V work vs token-sharding), runs causal attention for
those heads with per-query-chunk key-tile skipping (computes 20/32 of the
score rectangle instead of 24/32), then exchanges attention outputs with its
pair core so each core ends up with all 16 heads' outputs for its own 1024
tokens. The exchange is a 2-core ReduceScatter-with-zero-slots (AllToAll
needs a >4-core mesh): ccin rows are [token-half x rank-slot x feat]; a
host-provided 0/1 selector writes y into the own-rank slot and zeros into
the other, keeping the program SPMD-uniform, and RS(add) returns a
rank-stacked, uniformly-addressed result. One RS per group pipelines behind
the remaining attention groups. proj weights are host-permuted to match the
rank-stacked feature order. proj/LN2/MLP proceed token-locally; the host
concatenates the 8 output halves.

Numerics: all matmul operands bf16 (PSUM f32); residual path f32; softmax is
exp-then-multiplicative-{0,1}-mask (only diagonal key tiles are masked);
denominators ride as a ones-row in the V stationary; reciprocals via
reciprocal_approx_fast.
"""

from contextlib import ExitStack

import numpy as np
import ml_dtypes

import concourse.bacc as bacc
import concourse.bass as bass
import concourse.tile as tile
from concourse import mybir
from concourse.bass_utils import run_bass_kernel_spmd


F32 = mybir.dt.float32
F32R = mybir.dt.float32r
BF16 = mybir.dt.bfloat16
P = 128
B, T, C = 4, 2048, 1024
H, D = 16, 64
DFF = 4096
TOWN = 1024            # tokens owned per core (MLP phase)
HC = 512               # head-sliced feature dim per core (8 heads)
EPS = 1e-5
SCALE = D ** -0.5

KT_C = C // P          # 8 contraction tiles over C
FT_C = C // P          # 8 feature tiles over C
TT_FULL = T // P       # 16 token tiles (full seq)
NG = 4                 # head-pair groups per core (8 local heads)
NK2 = (2, 4, 6, 8)     # key units (256 keys) per query chunk qc
REPL = [[0, 1], [2, 3], [4, 5], [6, 7]]

Ident = mybir.ActivationFunctionType.Identity
Sqrt = mybir.ActivationFunctionType.Sqrt
Exp = mybir.ActivationFunctionType.Exp
Relu = mybir.ActivationFunctionType.Relu
ADD = mybir.AluOpType.add
MULT = mybir.AluOpType.mult
BYPASS = mybir.AluOpType.bypass


def _alloc(pool, n, shape, dt, prefix, **kw):
    return [
        pool.tile(list(shape), dt, tag=f"{prefix}{i}", name=f"{prefix}{i}", **kw)
        for i in range(n)
    ]


def _ln_b16(nc, x_loader, nblocks, dst, dst_off, g_col, b_col, eps_t, ones_b,
            st_ps, rowp, bcp, apl, prefix):
    """LayerNorm, feature-major, bf16 in/out (see v2 notes)."""
    for nb in range(nblocks):
        sl = slice(dst_off + nb * 512, dst_off + (nb + 1) * 512)
        xb = [x_loader(kt, nb) for kt in range(KT_C)]
        ssum = st_ps.tile([P, 512], F32, tag="ss", name=f"{prefix}ss{nb}")
        ssq = st_ps.tile([P, 512], F32, tag="sq", name=f"{prefix}sq{nb}")
        for kt in range(KT_C):
            nc.tensor.matmul(ssum, ones_b, xb[kt],
                             start=(kt == 0), stop=(kt == KT_C - 1))
        for kt in range(KT_C):
            sq = rowp.tile([P, 512], BF16, tag=f"sqt{kt}",
                           name=f"{prefix}sqt{nb}_{kt}", bufs=1)
            nc.vector.tensor_mul(out=sq, in0=xb[kt], in1=xb[kt])
            nc.tensor.matmul(ssq, ones_b, sq,
                             start=(kt == 0), stop=(kt == KT_C - 1))
        mu = rowp.tile([1, 512], F32, tag="mu", name=f"{prefix}mu{nb}")
        nc.scalar.mul(mu, ssum[0:1, :], 1.0 / C)
        msq = rowp.tile([1, 512], F32, tag="msq", name=f"{prefix}msq{nb}")
        nc.scalar.mul(msq, ssq[0:1, :], 1.0 / C)
        var = rowp.tile([1, 512], F32, tag="mu2", name=f"{prefix}var{nb}")
        nc.vector.tensor_mul(out=var, in0=mu, in1=mu)
        nc.vector.tensor_sub(out=var, in0=msq, in1=var)
        std = rowp.tile([1, 512], F32, tag="msq", name=f"{prefix}std{nb}")
        nc.scalar.activation(out=std, in_=var, func=Sqrt,
                             bias=eps_t[0:1, 0:1], scale=1.0)
        rs = rowp.tile([1, 512], F32, tag="mu2", name=f"{prefix}rs{nb}")
        nc.vector.reciprocal_approx_fast(out=rs, in_=std)
        mu16 = rowp.tile([1, 512], BF16, tag="mu16", name=f"{prefix}mu16{nb}")
        nc.vector.tensor_copy(out=mu16, in_=mu)
        rs16 = rowp.tile([1, 512], BF16, tag="rs16", name=f"{prefix}rs16{nb}")
        nc.vector.tensor_copy(out=rs16, in_=rs)
        mu_b = bcp.tile([P, 512], BF16, tag="mub", name=f"{prefix}mub{nb}")
        nc.gpsimd.partition_broadcast(mu_b, mu16)
        rs_b = bcp.tile([P, 512], BF16, tag="rsb", name=f"{prefix}rsb{nb}")
        nc.gpsimd.partition_broadcast(rs_b, rs16)
        for ft in range(FT_C):
            t = apl.tile([P, 512], BF16, tag=f"ap{ft}",
                         name=f"{prefix}ap{nb}_{ft}", bufs=2)
            nc.vector.tensor_sub(out=t, in0=xb[ft], in1=mu_b)
            nc.vector.tensor_mul(out=t, in0=t, in1=rs_b)
            nc.scalar.activation(out=dst[ft][:, sl], in_=t, func=Ident,
                                 bias=b_col[:, ft:ft + 1],
                                 scale=g_col[:, ft:ft + 1])


def build_nc():
    nc = bacc.Bacc()
    xT_b16 = nc.declare_dram_parameter("xT_b16", [C, T], BF16, isOutput=False)
    xTo_f32 = nc.declare_dram_parameter("xTo_f32", [C, TOWN], F32, isOutput=False)
    maskd = nc.declare_dram_parameter("maskd", [256, 1024], BF16, isOutput=False)
    selbc = nc.declare_dram_parameter("selbc", [P, 2], F32, isOutput=False)
    wq_s = nc.declare_dram_parameter("wq_s", [C, HC], BF16, isOutput=False)
    wk_s = nc.declare_dram_parameter("wk_s", [C, HC], BF16, isOutput=False)
    wv_s = nc.declare_dram_parameter("wv_s", [C, HC], BF16, isOutput=False)
    attn_bs = nc.declare_dram_parameter("attn_bs", [3 * HC], F32, isOutput=False)
    proj_wp = nc.declare_dram_parameter("proj_wp", [C, C], BF16, isOutput=False)
    proj_b = nc.declare_dram_parameter("proj_b", [C], F32, isOutput=False)
    ln1_g = nc.declare_dram_parameter("ln1_g", [C], F32, isOutput=False)
    ln1_b = nc.declare_dram_parameter("ln1_b", [C], F32, isOutput=False)
    ln2_g = nc.declare_dram_parameter("ln2_g", [C], F32, isOutput=False)
    ln2_b = nc.declare_dram_parameter("ln2_b", [C], F32, isOutput=False)
    fc1_w = nc.declare_dram_parameter("fc1_w", [C, DFF], BF16, isOutput=False)
    fc1_b = nc.declare_dram_parameter("fc1_b", [DFF], F32, isOutput=False)
    fc2_w = nc.declare_dram_parameter("fc2_w", [DFF, C], BF16, isOutput=False)
    fc2_b = nc.declare_dram_parameter("fc2_b", [C], F32, isOutput=False)
    out = nc.declare_dram_parameter("out", [C, TOWN], F32, isOutput=True)

    with tile.TileContext(nc) as tc, ExitStack() as top:
        const = top.enter_context(tc.tile_pool(name="const", bufs=1))
        eps_t = const.tile([P, 1], F32, name="eps_t")
        nc.vector.memset(eps_t, EPS)
        ones_f = const.tile([P, 1], F32, name="ones_f")
        nc.vector.memset(ones_f, 1.0)
        ones_b = const.tile([P, P], BF16, name="ones_b")
        nc.vector.memset(ones_b, 1.0)
        ones_r = const.tile([P, 1], F32R, name="ones_r")
        nc.vector.tensor_copy(out=ones_r, in_=ones_f)
        ln1g_t = const.tile([P, FT_C], F32, name="ln1g_t")
        ln1b_t = const.tile([P, FT_C], F32, name="ln1b_t")
        ln2g_t = const.tile([P, FT_C], F32, name="ln2g_t")
        ln2b_t = const.tile([P, FT_C], F32, name="ln2b_t")
        abq_t = const.tile([P, NG], F32, name="abq_t")
        abk_t = const.tile([P, NG], F32, name="abk_t")
        projb_t = const.tile([P, FT_C], F32, name="projb_t")
        fc2b_t = const.tile([P, FT_C], F32, name="fc2b_t")
        fc1b_t = const.tile([P, DFF // P], F32, name="fc1b_t")
        bv_bc = const.tile([P, HC], BF16, name="bv_bc")

        # K/V/Q live from phase A until the end of attention
        s1 = ExitStack()
        qkvp = s1.enter_context(tc.tile_pool(name="qkvp", bufs=1))
        qT = _alloc(qkvp, NG, [P, T], BF16, "qT")
        kT = _alloc(qkvp, NG, [P, T], BF16, "kT")
        vG = qkvp.tile([P, TT_FULL * 8 * 65], BF16, name="vG")
        vG4 = vG.rearrange("p (t h x) -> p t h x", t=TT_FULL, h=8)

        # ---- Phase A: LN1 (full seq) + QKV projections (own heads) ----
        with ExitStack() as cA:
            st_ps = cA.enter_context(
                tc.tile_pool(name="st_ps", bufs=1, space="PSUM"))
            mm_ps = cA.enter_context(
                tc.tile_pool(name="mm_ps", bufs=1, space="PSUM"))
            rowp = cA.enter_context(tc.tile_pool(name="rowp", bufs=1))
            bcp = cA.enter_context(tc.tile_pool(name="bcp", bufs=2))
            apl = cA.enter_context(tc.tile_pool(name="apl", bufs=1))
            hp = cA.enter_context(tc.tile_pool(name="hp", bufs=1))
            wp = cA.enter_context(tc.tile_pool(name="wp", bufs=1))
            lnp = cA.enter_context(tc.tile_pool(name="lnp", bufs=1))

            hTf = _alloc(hp, FT_C, [P, T], BF16, "hTf")

            _dmaq = [nc.sync, nc.scalar, nc.gpsimd]

            # x DMAs go out before any const/weight descriptors occupy the
            # queues so the first LN stats matmul isn't gated on queue drain.
            xf_pre = [[None] * KT_C for _ in range(4)]
            for nb in range(4):
                for kt in range(KT_C):
                    t = lnp.tile([P, 512], BF16, tag=f"x{kt}",
                                 name=f"xf{kt}_{nb}", bufs=3)
                    # block 0 rides only the two hardware DGE queues so the
                    # first LN stats matmuls aren't paced by the software
                    # (gpsimd) queue
                    q = ([nc.sync, nc.scalar][kt % 2] if nb == 0
                         else _dmaq[kt % 3])
                    q.dma_start(
                        out=t, in_=xT_b16[kt * P:(kt + 1) * P,
                                          nb * 512:(nb + 1) * 512])
                    xf_pre[nb][kt] = t

            # deferred const loads (needed from the LN apply onwards)
            nc.scalar.dma_start(out=ln1g_t,
                                in_=ln1_g.rearrange("(f p) -> p f", p=P))
            nc.scalar.dma_start(out=ln1b_t,
                                in_=ln1_b.rearrange("(f p) -> p f", p=P))
            nc.sync.dma_start(out=abq_t,
                              in_=attn_bs[0:HC].rearrange("(g p) -> p g", p=P))
            nc.sync.dma_start(out=abk_t,
                              in_=attn_bs[HC:2 * HC].rearrange("(g p) -> p g",
                                                               p=P))
            nc.scalar.dma_start(out=ln2g_t,
                                in_=ln2_g.rearrange("(f p) -> p f", p=P))
            nc.scalar.dma_start(out=ln2b_t,
                                in_=ln2_b.rearrange("(f p) -> p f", p=P))
            nc.scalar.dma_start(out=projb_t,
                                in_=proj_b.rearrange("(f p) -> p f", p=P))
            nc.scalar.dma_start(out=fc2b_t,
                                in_=fc2_b.rearrange("(f p) -> p f", p=P))
            nc.scalar.dma_start(out=fc1b_t,
                                in_=fc1_b.rearrange("(f p) -> p f", p=P))

            # streamed head-sliced weight tiles [P, 512]
            def wload(w_dram, idx, nm):
                w = wp.tile([P, HC], BF16, tag=f"w{idx}", name=nm, bufs=2)
                _dmaq[idx % 2].dma_start(
                    out=w, in_=w_dram[idx * P:(idx + 1) * P, :])
                return w

            wq = [wload(wq_s, kt, f"wq{kt}") for kt in range(KT_C)]

            _ln_b16(nc, lambda kt, nb: xf_pre[nb][kt][:, :], 4, hTf, 0,
                    ln1g_t, ln1b_t, eps_t, ones_b,
                    st_ps, rowp, bcp, apl, "lf")

            # Q for all tokens, own heads
            for g in range(NG):
                qps = [mm_ps.tile([P, 512], F32, tag=f"mm{nb}",
                                  name=f"qps{g}_{nb}") for nb in range(4)]
                for kt in range(KT_C):
                    for nb in range(4):
                        nc.tensor.matmul(
                            qps[nb], wq[kt][:, g * P:(g + 1) * P],
                            hTf[kt][:, nb * 512:(nb + 1) * 512],
                            start=(kt == 0), stop=(kt == KT_C - 1))
                for nb in range(4):
                    nc.vector.tensor_scalar_add(
                        out=qT[g][:, nb * 512:(nb + 1) * 512], in0=qps[nb],
                        scalar1=abq_t[:, g:g + 1])

            # K for all tokens, own heads
            wk = [wload(wk_s, kt, f"wk{kt}") for kt in range(KT_C)]
            for g in range(NG):
                kps = [mm_ps.tile([P, 512], F32, tag=f"mm{nb}",
                                  name=f"kps{g}_{nb}") for nb in range(4)]
                for kt in range(KT_C):
                    for nb in range(4):
                        nc.tensor.matmul(
                            kps[nb], wk[kt][:, g * P:(g + 1) * P],
                            hTf[kt][:, nb * 512:(nb + 1) * 512],
                            start=(kt == 0), stop=(kt == KT_C - 1))
                for nb in range(4):
                    nc.vector.tensor_scalar_add(
                        out=kT[g][:, nb * 512:(nb + 1) * 512], in0=kps[nb],
                        scalar1=abk_t[:, g:g + 1])

            # V rows (token-major), own heads
            nc.gpsimd.memset(vG4[:, :, :, 64:65], 1.0)  # softmax denom ones
            abv = attn_bs[2 * HC:3 * HC]
            nc.gpsimd.dma_start(
                out=bv_bc,
                in_=bass.AP(tensor=abv.tensor, offset=abv.offset,
                            ap=[[0, P]] + list(abv.ap[-1:])))
            wv = [wload(wv_s, kt, f"wv{kt}") for kt in range(KT_C)]
            for tt in range(TT_FULL):
                vps = mm_ps.tile([P, HC], F32, tag=f"mv{tt % 2}",
                                 name=f"vps{tt}")
                for kt in range(KT_C):
                    nc.tensor.matmul(
                        vps, hTf[kt][:, tt * P:(tt + 1) * P], wv[kt],
                        start=(kt == 0), stop=(kt == KT_C - 1))
                nc.vector.tensor_add(
                    out=vG4[:, tt, :, 0:64],
                    in0=vps.rearrange("p (h d) -> p h d", d=64),
                    in1=bv_bc.rearrange("p (h d) -> p h d", d=64))

        # ---- Phase B: attention (own heads, causal-tile-skipped) ----
        sM = ExitStack()
        attnp = sM.enter_context(tc.tile_pool(name="attnp", bufs=1, side="right"))
        attnT = _alloc(attnp, NG, [P, T], BF16, "attnT")
        pw = _alloc(attnp, KT_C, [P, C], BF16, "pw")
        xo32 = _alloc(attnp, FT_C, [P, TOWN], F32, "xo32")

        ccp = top.enter_context(tc.tile_pool(name="ccp", bufs=1, space="DRAM"))
        ccin = _alloc(ccp, NG, [4 * P, 1024], BF16, "ccin")
        ccout = _alloc(ccp, NG, [2 * P, 1024], BF16, "ccout")
        selt = const.tile([P, 2], F32, name="selt")
        nc.sync.dma_start(out=selt, in_=selbc[:, :])

        with ExitStack() as cB:
            mpool = cB.enter_context(tc.tile_pool(name="mpool", bufs=1))
            md = _alloc(mpool, 2, [P, 1024], BF16, "md")
            for k2 in range(2):
                nc.sync.dma_start(out=md[k2], in_=maskd[k2 * P:(k2 + 1) * P, :])
            for kt in range(KT_C):
                nc.sync.dma_start(out=pw[kt],
                                  in_=proj_wp[kt * P:(kt + 1) * P, :])
                nc.scalar.dma_start(out=xo32[kt],
                                    in_=xTo_f32[kt * P:(kt + 1) * P, :])

            sc_ps = cB.enter_context(
                tc.tile_pool(name="sc_ps", bufs=2, space="PSUM"))
            y_ps_pool = cB.enter_context(
                tc.tile_pool(name="y_ps_pool", bufs=1, space="PSUM"))
            ppool = cB.enter_context(tc.tile_pool(name="ppool", bufs=6))
            npool = cB.enter_context(tc.tile_pool(name="npool", bufs=2))
            ccsb = cB.enter_context(tc.tile_pool(name="ccsb", bufs=2))

            for g in range(NG):
                # two qc-pair passes: (0,1) over k2<4, (2,3) over k2<8
                for qa, qb in ((0, 1), (2, 3)):
                    y_ps = {
                        (qc, hh): y_ps_pool.tile(
                            [65, 512], F32, tag=f"y{qc % 2}{hh}",
                            name=f"y{g}_{qc}_{hh}")
                        for qc in (qa, qb) for hh in range(2)
                    }
                    for k2 in range(NK2[qb]):
                        for hh in range(2):
                            hsl = slice(64 * hh, 64 * (hh + 1))
                            scs = {}
                            if k2 < NK2[qa]:
                                scs[qa] = sc_ps.tile(
                                    [P, 1024], F32, tag="sc",
                                    name=f"sc{g}_{qa}_{k2}_{hh}")
                            scs[qb] = sc_ps.tile(
                                [P, 1024], F32, tag="sc",
                                name=f"sc{g}_{qb}_{k2}_{hh}")
                            for j in range(2):
                                kt = 2 * k2 + j
                                ksl = kT[g][hsl, kt * P:(kt + 1) * P]
                                for qc in scs:
                                    nc.tensor.matmul(
                                        scs[qc][:, j * 512:(j + 1) * 512],
                                        ksl,
                                        qT[g][hsl, qc * 512:(qc + 1) * 512],
                                        start=True, stop=True,
                                        tile_position=(64 * hh, 0))
                            pts = {}
                            for qc in scs:
                                pts[qc] = ppool.tile(
                                    [P, 1024], BF16, tag="pt",
                                    name=f"p{g}_{qc}_{k2}_{hh}")
                                nc.scalar.activation(out=pts[qc], in_=scs[qc],
                                                     func=Exp, scale=SCALE)
                                # mask diagonal tiles: qc's diagonal sits at
                                # k2 in {2qc, 2qc+1}
                                if k2 == 2 * qc:
                                    nc.vector.tensor_mul(
                                        out=pts[qc], in0=pts[qc], in1=md[0])
                                elif k2 == 2 * qc + 1:
                                    nc.vector.tensor_mul(
                                        out=pts[qc], in0=pts[qc], in1=md[1])
                            for j in range(2):
                                kt = 2 * k2 + j
                                vsl = vG4[:, kt, 2 * g + hh, :]
                                for qc in pts:
                                    nc.tensor.matmul(
                                        y_ps[(qc, hh)],
                                        vsl,
                                        pts[qc][:, j * 512:(j + 1) * 512],
                                        start=(kt == 0),
                                        stop=(kt == 2 * NK2[qc] - 1))
                    for qc in (qa, qb):
                        for hh in range(2):
                            dn = npool.tile([1, 512], F32, tag="dn",
                                            name=f"dn{g}_{qc}_{hh}")
                            nc.vector.tensor_copy(
                                out=dn, in_=y_ps[(qc, hh)][64:65, :])
                            r = npool.tile([1, 512], F32, tag="r",
                                           name=f"r{g}_{qc}_{hh}")
                            nc.vector.reciprocal_approx_fast(out=r, in_=dn)
                            rb = npool.tile([64, 512], F32, tag="rb",
                                            name=f"rb{g}_{qc}_{hh}")
                            nc.gpsimd.partition_broadcast(rb, r[0:1, :])
                            nc.vector.tensor_mul(
                                out=attnT[g][64 * hh:64 * (hh + 1),
                                             qc * 512:(qc + 1) * 512],
                                in0=y_ps[(qc, hh)][0:64, :], in1=rb)
                # Exchange this group's outputs with the pair core via
                # ReduceScatter-with-zero-slots: ccin rows are
                # [half h (2) x rank-slot s (2) x feat (128)]; each core
                # writes y into slot s==rank (selbc data picks the slot,
                # keeping the program SPMD-uniform) and zeros elsewhere.
                # RS(add) hands rank j rows [s*128+f] = rank s's heads for
                # token-half j - rank-stacked, uniform to read.
                ts = {}
                for s in range(2):
                    ts[s] = ccsb.tile([P, T], BF16, tag=f"ts{s}",
                                      name=f"ts{g}_{s}")
                    nc.vector.tensor_scalar_mul(
                        out=ts[s], in0=attnT[g], scalar1=selt[:, s:s + 1])
                for hhalf in range(2):
                    for s in range(2):
                        nc.sync.dma_start(
                            out=ccin[g][hhalf * 2 * P + s * P:
                                        hhalf * 2 * P + (s + 1) * P, :],
                            in_=ts[s][:, hhalf * 1024:(hhalf + 1) * 1024])
                nc.gpsimd.collective_compute(
                    "ReduceScatter", ADD, replica_groups=REPL,
                    ins=[ccin[g].opt()], outs=[ccout[g].opt()])

        s1.close()   # free qT/kT/vG

        # ---- gather exchanged attention rows; proj + residual -> x2T ----
        x2p = top.enter_context(tc.tile_pool(name="x2p", bufs=1))
        aT = _alloc(x2p, KT_C, [P, TOWN], BF16, "aT")
        x2T = _alloc(x2p, FT_C, [P, TOWN], F32R, "x2T")
        h2T = _alloc(x2p, FT_C, [P, TOWN], BF16, "h2T")
        for kt in range(KT_C):
            g, r = kt // 2, kt % 2
            _dmaq[kt % 3].dma_start(out=aT[kt],
                                    in_=ccout[g][r * P:(r + 1) * P, :])

        # prefetch the first half of fc1_w during proj/LN2
        w1p = top.enter_context(tc.tile_pool(name="w1p", bufs=1))

        def w1load(dh, kt):
            w = w1p.tile([P, DFF // 2], BF16, tag=f"w1_{kt}",
                         name=f"w1h{dh}_{kt}", bufs=1)
            nc.sync.dma_start(
                out=w,
                in_=fc1_w[kt * P:(kt + 1) * P,
                          dh * (DFF // 2):(dh + 1) * (DFF // 2)])
            return w

        w1h0 = [w1load(0, kt) for kt in range(KT_C)]

        with ExitStack() as cC:
            pj_ps = cC.enter_context(
                tc.tile_pool(name="pj_ps", bufs=2, space="PSUM"))
            st2 = cC.enter_context(
                tc.tile_pool(name="st2", bufs=2, space="PSUM"))
            rowp2 = cC.enter_context(tc.tile_pool(name="rowp2", bufs=1))
            bcp2 = cC.enter_context(tc.tile_pool(name="bcp2", bufs=2))
            apl2 = cC.enter_context(tc.tile_pool(name="apl2", bufs=1))

            # proj and LN2 interleaved per token-half: LN2(nb=0) overlaps
            # proj(nb=1), so fc1 only waits on the second half's apply.
            for nb in range(2):
                sl = slice(nb * 512, (nb + 1) * 512)
                for ft in range(FT_C):
                    pps = pj_ps.tile([P, 512], F32, tag=f"pj{ft % 2}",
                                     name=f"pps{ft}_{nb}")
                    for kt in range(KT_C):
                        nc.tensor.matmul(
                            pps, pw[kt][:, ft * P:(ft + 1) * P],
                            aT[kt][:, sl],
                            start=(kt == 0), stop=(kt == KT_C - 1))
                    nc.vector.scalar_tensor_tensor(
                        out=x2T[ft][:, sl], in0=pps,
                        scalar=projb_t[:, ft:ft + 1],
                        in1=xo32[ft][:, sl], op0=ADD, op1=ADD)

                # LN2 for this half, in f32 (x2T is the precise residual)
                ssum = st2.tile([1, 512], F32, tag="ss2", name=f"l2ss{nb}")
                ssq = st2.tile([1, 512], F32, tag="sq2", name=f"l2sq{nb}")
                for kt in range(KT_C):
                    nc.tensor.matmul(ssum, ones_r, x2T[kt][:, sl],
                                     start=(kt == 0), stop=(kt == KT_C - 1))
                for kt in range(KT_C):
                    sq = rowp2.tile([P, 512], F32R, tag="sqt",
                                    name=f"l2sqt{nb}_{kt}", bufs=3)
                    nc.vector.tensor_mul(out=sq, in0=x2T[kt][:, sl],
                                         in1=x2T[kt][:, sl])
                    nc.tensor.matmul(ssq, ones_r, sq,
                                     start=(kt == 0), stop=(kt == KT_C - 1))
                mu = rowp2.tile([1, 512], F32, tag="mu", name=f"l2mu{nb}")
                nc.scalar.mul(mu, ssum[0:1, :], 1.0 / C)
                msq = rowp2.tile([1, 512], F32, tag="msq", name=f"l2msq{nb}")
                nc.scalar.mul(msq, ssq[0:1, :], 1.0 / C)
                var = rowp2.tile([1, 512], F32, tag="var", name=f"l2var{nb}")
                nc.vector.tensor_mul(out=var, in0=mu, in1=mu)
                nc.vector.tensor_sub(out=var, in0=msq, in1=var)
                std = rowp2.tile([1, 512], F32, tag="std", name=f"l2std{nb}")
                nc.scalar.activation(out=std, in_=var, func=Sqrt,
                                     bias=eps_t[0:1, 0:1], scale=1.0)
                rs = rowp2.tile([1, 512], F32, tag="rs", name=f"l2rs{nb}")
                nc.vector.reciprocal_approx_fast(out=rs, in_=std)
                mu_b = bcp2.tile([P, 512], F32, tag="mub", name=f"l2mub{nb}")
                nc.gpsimd.partition_broadcast(mu_b, mu)
                rs_b = bcp2.tile([P, 512], F32, tag="rsb", name=f"l2rsb{nb}")
                nc.gpsimd.partition_broadcast(rs_b, rs)
                for ft in range(FT_C):
                    t = apl2.tile([P, 512], F32, tag="ap",
                                  name=f"l2ap{nb}_{ft}", bufs=3)
                    nc.vector.tensor_sub(out=t,
                                         in0=x2T[ft][:, sl].bitcast(F32),
                                         in1=mu_b)
                    nc.vector.tensor_mul(out=t, in0=t, in1=rs_b)
                    nc.scalar.activation(out=h2T[ft][:, sl], in_=t,
                                         func=Ident,
                                         bias=ln2b_t[:, ft:ft + 1],
                                         scale=ln2g_t[:, ft:ft + 1])
        sM.close()   # free attnT/pw/xo32

        # ---- Phase C: MLP ----
        with ExitStack() as cD:
            h1p = cD.enter_context(tc.tile_pool(name="h1p", bufs=1))
            h1 = _alloc(h1p, DFF // P, [P, TOWN], BF16, "h1")
            with ExitStack() as cD1:
                f1_ps = cD1.enter_context(
                    tc.tile_pool(name="f1_ps", bufs=2, space="PSUM"))
                for dh in range(2):
                    w1h = w1h0 if dh == 0 else [w1load(1, kt)
                                                for kt in range(KT_C)]
                    for dtl in range(16):
                        dt = dh * 16 + dtl
                        fps = [f1_ps.tile([P, 512], F32, tag=f"f1{nb}",
                                          name=f"fps{dt}_{nb}")
                               for nb in range(2)]
                        for kt in range(KT_C):
                            for nb in range(2):
                                nc.tensor.matmul(
                                    fps[nb], w1h[kt][:, dtl * P:(dtl + 1) * P],
                                    h2T[kt][:, nb * 512:(nb + 1) * 512],
                                    start=(kt == 0), stop=(kt == KT_C - 1))
                        for nb in range(2):
                            nc.scalar.activation(
                                out=h1[dt][:, nb * 512:(nb + 1) * 512],
                                in_=fps[nb], func=Relu,
                                bias=fc1b_t[:, dt:dt + 1], scale=1.0)

            with ExitStack() as cD2:
                f2_ps = cD2.enter_context(
                    tc.tile_pool(name="f2_ps", bufs=1, space="PSUM"))
                w2p = cD2.enter_context(tc.tile_pool(name="w2p", bufs=1))
                opool = cD2.enter_context(tc.tile_pool(name="opool", bufs=3))
                for fb, (f0, nf) in enumerate([(0, 4), (4, 2), (6, 2)]):
                    fp2 = [f2_ps.tile([P, TOWN], F32, tag=f"f2_{i}",
                                      name=f"fp2_{fb}_{i}")
                           for i in range(nf)]
                    for k8 in range(DFF // P):
                        w2t = w2p.tile([P, nf * P], BF16, tag=f"w2_{nf}",
                                       name=f"w2_{fb}_{k8}", bufs=6)
                        nc.sync.dma_start(
                            out=w2t, in_=fc2_w[k8 * P:(k8 + 1) * P,
                                               f0 * P:(f0 + nf) * P])
                        for i in range(nf):
                            for nb in range(2):
                                nc.tensor.matmul(
                                    fp2[i][:, nb * 512:(nb + 1) * 512],
                                    w2t[:, i * P:(i + 1) * P],
                                    h1[k8][:, nb * 512:(nb + 1) * 512],
                                    start=(k8 == 0), stop=(k8 == DFF // P - 1))
                    for i in range(nf):
                        ft = f0 + i
                        o = opool.tile([P, TOWN], F32, tag="o", name=f"o{ft}")
                        nc.vector.scalar_tensor_tensor(
                            out=o, in0=fp2[i], scalar=fc2b_t[:, ft:ft + 1],
                            in1=x2T[ft].bitcast(F32), op0=ADD, op1=ADD)
                        nc.sync.dma_start(out=out[ft * P:(ft + 1) * P, :],
                                          in_=o)

    nc.compile()
    return nc


_NC_CACHE = None


def _get_nc():
    global _NC_CACHE
    if _NC_CACHE is None:
        _NC_CACHE = build_nc()
    return _NC_CACHE


def _make_diag_mask():
    # [512 keys, 512 q] lower-triangular diag block, pair-packed to
    # [256, 1024]: row-block k2 (128 rows) holds [mask(kt=2k2) | mask(2k2+1)]
    k = np.arange(512, dtype=np.int64)[:, None]
    q = np.arange(512, dtype=np.int64)[None, :]
    m = (k <= q).astype(np.float32)
    return np.ascontiguousarray(
        m.reshape(2, 2, 128, 512).transpose(0, 2, 1, 3).reshape(256, 1024))


def _run(inputs, trace=False):
    nc = _get_nc()
    bf = ml_dtypes.bfloat16
    xs = {k: np.ascontiguousarray(np.asarray(v), dtype=np.float32)
          for k, v in inputs.items()}
    x = xs["x"]
    xT = {b: np.ascontiguousarray(x[b].T) for b in range(B)}
    md = _make_diag_mask().astype(bf)
    aw, ab = xs["attn_w"], xs["attn_b"]
    # proj rows permuted to the AllToAll rank-stacked feature order:
    # row (256g + 128r + d) <- global feature (8r + 2g)*64 + d
    perm = np.concatenate([
        np.arange((8 * r + 2 * g) * 64, (8 * r + 2 * g + 2) * 64)
        for g in range(NG) for r in range(2)])
    proj_wp = np.ascontiguousarray(xs["proj_w"][perm, :].astype(bf))
    w1 = np.ascontiguousarray(xs["fc1_w"].astype(bf))
    w2 = np.ascontiguousarray(xs["fc2_w"].astype(bf))
    in_maps = []
    for c in range(8):
        b, j = divmod(c, 2)
        hsl = slice(HC * j, HC * (j + 1))
        sel = np.zeros((P, 2), np.float32)
        sel[:, j] = 1.0
        in_maps.append({
            "xT_b16": np.ascontiguousarray(xT[b].astype(bf)),
            "xTo_f32": np.ascontiguousarray(xT[b][:, TOWN * j:TOWN * (j + 1)]),
            "maskd": md,
            "selbc": sel,
            "wq_s": np.ascontiguousarray(aw[:, 0:C][:, hsl].astype(bf)),
            "wk_s": np.ascontiguousarray(aw[:, C:2 * C][:, hsl].astype(bf)),
            "wv_s": np.ascontiguousarray(aw[:, 2 * C:3 * C][:, hsl].astype(bf)),
            "attn_bs": np.ascontiguousarray(np.concatenate(
                [ab[0:C][hsl], ab[C:2 * C][hsl], ab[2 * C:3 * C][hsl]])),
            "proj_wp": proj_wp, "proj_b": xs["proj_b"],
            "ln1_g": xs["ln1_g"], "ln1_b": xs["ln1_b"],
            "ln2_g": xs["ln2_g"], "ln2_b": xs["ln2_b"],
            "fc1_w": w1, "fc1_b": xs["fc1_b"],
            "fc2_w": w2, "fc2_b": xs["fc2_b"],
        })
    res = run_bass_kernel_spmd(nc, in_maps, list(range(8)), trace=trace)
    full = np.empty((B, T, C), dtype=np.float32)
    for c in range(8):
        b, j = divmod(c, 2)
        o = res.results[c]["out"]            # [C, TOWN] feature-major
        full[b, TOWN * j:TOWN * (j + 1)] = o.T
    return full, res.exec_time_ns


def kernel(**inputs):
    out, _ = _run(inputs, trace=False)
    return out


# revision 63
# speedup vs baseline: 1.0646x; 1.0646x over previous
"""Trainium2 Bass kernel for a dense transformer block (nn_Block_30262339567972).

Full inputs in, full outputs out. Head-sharded across 8 NeuronCores with one
ReduceScatter per head-pair group:

  core c = 2*b + j  (batch b, half j) owns
    - attention: GLOBAL heads [8j, 8j+8) over the full 2048-token sequence
    - MLP/residual/output: tokens [1024j, 1024(j+1)) of batch b

Each core computes LN1 for the full sequence once, projects Q/K/V only for
its 8 heads (halving K/V work vs token-sharding), runs causal attention for
those heads with per-query-chunk key-tile skipping (computes 20/32 of the
score rectangle instead of 24/32), then exchanges attention outputs with its
pair core so each core ends up with all 16 heads' outputs for its own 1024
tokens. The exchange is a 2-core ReduceScatter-with-zero-slots (AllToAll
needs a >4-core mesh): ccin rows are [token-half x rank-slot x feat]; a
host-provided 0/1 selector writes y into the own-rank slot and zeros into
the other, keeping the program SPMD-uniform, and RS(add) returns a
rank-stacked, uniformly-addressed result. One RS per group pipelines behind
the remaining attention groups. proj weights are host-permuted to match the
rank-stacked feature order. proj/LN2/MLP proceed token-locally; the host
concatenates the 8 output halves.

Numerics: all matmul operands bf16 (PSUM f32); residual path f32; softmax is
exp-then-multiplicative-{0,1}-mask (only diagonal key tiles are masked);
denominators ride as a ones-row in the V stationary; reciprocals via
reciprocal_approx_fast.
"""

from contextlib import ExitStack

import numpy as np
import ml_dtypes

import concourse.bacc as bacc
import concourse.bass as bass
import concourse.tile as tile
from concourse import mybir
from concourse.bass_utils import run_bass_kernel_spmd


F32 = mybir.dt.float32
F32R = mybir.dt.float32r
BF16 = mybir.dt.bfloat16
P = 128
B, T, C = 4, 2048, 1024
H, D = 16, 64
DFF = 4096
TOWN = 1024            # tokens owned per core (MLP phase)
HC = 512               # head-sliced feature dim per core (8 heads)
EPS = 1e-5
SCALE = D ** -0.5

KT_C = C // P          # 8 contraction tiles over C
FT_C = C // P          # 8 feature tiles over C
TT_FULL = T // P       # 16 token tiles (full seq)
NG = 4                 # head-pair groups per core (8 local heads)
NK2 = (2, 4, 6, 8)     # key units (256 keys) per query chunk qc
REPL = [[0, 1], [2, 3], [4, 5], [6, 7]]

Ident = mybir.ActivationFunctionType.Identity
Sqrt = mybir.ActivationFunctionType.Sqrt
Exp = mybir.ActivationFunctionType.Exp
Relu = mybir.ActivationFunctionType.Relu
ADD = mybir.AluOpType.add
MULT = mybir.AluOpType.mult
BYPASS = mybir.AluOpType.bypass


def _alloc(pool, n, shape, dt, prefix, **kw):
    return [
        pool.tile(list(shape), dt, tag=f"{prefix}{i}", name=f"{prefix}{i}", **kw)
        for i in range(n)
    ]


def _ln_b16(nc, x_loader, nblocks, dst, dst_off, g_col, b_col, eps_t, ones_b,
            st_ps, rowp, bcp, apl, prefix):
    """LayerNorm, feature-major, bf16 in/out (see v2 notes)."""
    for nb in range(nblocks):
        sl = slice(dst_off + nb * 512, dst_off + (nb + 1) * 512)
        xb = [x_loader(kt, nb) for kt in range(KT_C)]
        ssum = st_ps.tile([P, 512], F32, tag="ss", name=f"{prefix}ss{nb}")
        ssq = st_ps.tile([P, 512], F32, tag="sq", name=f"{prefix}sq{nb}")
        for kt in range(KT_C):
            nc.tensor.matmul(ssum, ones_b, xb[kt],
                             start=(kt == 0), stop=(kt == KT_C - 1))
        for kt in range(KT_C):
            sq = rowp.tile([P, 512], BF16, tag=f"sqt{kt}",
                           name=f"{prefix}sqt{nb}_{kt}", bufs=1)
            nc.vector.tensor_mul(out=sq, in0=xb[kt], in1=xb[kt])
            nc.tensor.matmul(ssq, ones_b, sq,
                             start=(kt == 0), stop=(kt == KT_C - 1))
        mu = rowp.tile([1, 512], F32, tag="mu", name=f"{prefix}mu{nb}")
        nc.scalar.mul(mu, ssum[0:1, :], 1.0 / C)
        msq = rowp.tile([1, 512], F32, tag="msq", name=f"{prefix}msq{nb}")
        nc.scalar.mul(msq, ssq[0:1, :], 1.0 / C)
        var = rowp.tile([1, 512], F32, tag="mu2", name=f"{prefix}var{nb}")
        nc.vector.tensor_mul(out=var, in0=mu, in1=mu)
        nc.vector.tensor_sub(out=var, in0=msq, in1=var)
        std = rowp.tile([1, 512], F32, tag="msq", name=f"{prefix}std{nb}")
        nc.scalar.activation(out=std, in_=var, func=Sqrt,
                             bias=eps_t[0:1, 0:1], scale=1.0)
        rs = rowp.tile([1, 512], F32, tag="mu2", name=f"{prefix}rs{nb}")
        nc.vector.reciprocal_approx_fast(out=rs, in_=std)
        mu16 = rowp.tile([1, 512], BF16, tag="mu16", name=f"{prefix}mu16{nb}")
        nc.vector.tensor_copy(out=mu16, in_=mu)
        rs16 = rowp.tile([1, 512], BF16, tag="rs16", name=f"{prefix}rs16{nb}")
        nc.vector.tensor_copy(out=rs16, in_=rs)
        mu_b = bcp.tile([P, 512], BF16, tag="mub", name=f"{prefix}mub{nb}")
        nc.gpsimd.partition_broadcast(mu_b, mu16)
        rs_b = bcp.tile([P, 512], BF16, tag="rsb", name=f"{prefix}rsb{nb}")
        nc.gpsimd.partition_broadcast(rs_b, rs16)
        for ft in range(FT_C):
            t = apl.tile([P, 512], BF16, tag=f"ap{ft}",
                         name=f"{prefix}ap{nb}_{ft}", bufs=2)
            nc.vector.tensor_sub(out=t, in0=xb[ft], in1=mu_b)
            nc.vector.tensor_mul(out=t, in0=t, in1=rs_b)
            nc.scalar.activation(out=dst[ft][:, sl], in_=t, func=Ident,
                                 bias=b_col[:, ft:ft + 1],
                                 scale=g_col[:, ft:ft + 1])


def build_nc():
    nc = bacc.Bacc()
    xT_b16 = nc.declare_dram_parameter("xT_b16", [C, T], BF16, isOutput=False)
    xTo_f32 = nc.declare_dram_parameter("xTo_f32", [C, TOWN], F32, isOutput=False)
    maskd = nc.declare_dram_parameter("maskd", [256, 1024], BF16, isOutput=False)
    selbc = nc.declare_dram_parameter("selbc", [P, 2], F32, isOutput=False)
    wq_s = nc.declare_dram_parameter("wq_s", [C, HC], BF16, isOutput=False)
    wk_s = nc.declare_dram_parameter("wk_s", [C, HC], BF16, isOutput=False)
    wv_s = nc.declare_dram_parameter("wv_s", [C, HC], BF16, isOutput=False)
    attn_bs = nc.declare_dram_parameter("attn_bs", [3 * HC], F32, isOutput=False)
    proj_wp = nc.declare_dram_parameter("proj_wp", [C, C], BF16, isOutput=False)
    proj_b = nc.declare_dram_parameter("proj_b", [C], F32, isOutput=False)
    ln1_g = nc.declare_dram_parameter("ln1_g", [C], F32, isOutput=False)
    ln1_b = nc.declare_dram_parameter("ln1_b", [C], F32, isOutput=False)
    ln2_g = nc.declare_dram_parameter("ln2_g", [C], F32, isOutput=False)
    ln2_b = nc.declare_dram_parameter("ln2_b", [C], F32, isOutput=False)
    fc1_w = nc.declare_dram_parameter("fc1_w", [C, DFF], BF16, isOutput=False)
    fc1_b = nc.declare_dram_parameter("fc1_b", [DFF], F32, isOutput=False)
    fc2_w = nc.declare_dram_parameter("fc2_w", [DFF, C], BF16, isOutput=False)
    fc2_b = nc.declare_dram_parameter("fc2_b", [C], F32, isOutput=False)
    out = nc.declare_dram_parameter("out", [C, TOWN], F32, isOutput=True)

    with tile.TileContext(nc) as tc, ExitStack() as top:
        const = top.enter_context(tc.tile_pool(name="const", bufs=1))
        eps_t = const.tile([P, 1], F32, name="eps_t")
        nc.vector.memset(eps_t, EPS)
        ones_f = const.tile([P, 1], F32, name="ones_f")
        nc.vector.memset(ones_f, 1.0)
        ones_b = const.tile([P, P], BF16, name="ones_b")
        nc.vector.memset(ones_b, 1.0)
        ones_r = const.tile([P, 1], F32R, name="ones_r")
        nc.vector.tensor_copy(out=ones_r, in_=ones_f)
        ln1g_t = const.tile([P, FT_C], F32, name="ln1g_t")
        ln1b_t = const.tile([P, FT_C], F32, name="ln1b_t")
        ln2g_t = const.tile([P, FT_C], F32, name="ln2g_t")
        ln2b_t = const.tile([P, FT_C], F32, name="ln2b_t")
        abq_t = const.tile([P, NG], F32, name="abq_t")
        abk_t = const.tile([P, NG], F32, name="abk_t")
        projb_t = const.tile([P, FT_C], F32, name="projb_t")
        fc2b_t = const.tile([P, FT_C], F32, name="fc2b_t")
        fc1b_t = const.tile([P, DFF // P], F32, name="fc1b_t")
        bv_bc = const.tile([P, HC], BF16, name="bv_bc")

        # K/V/Q live from phase A until the end of attention
        s1 = ExitStack()
        qkvp = s1.enter_context(tc.tile_pool(name="qkvp", bufs=1))
        qT = _alloc(qkvp, NG, [P, T], BF16, "qT")
        kT = _alloc(qkvp, NG, [P, T], BF16, "kT")
        vG = qkvp.tile([P, TT_FULL * 8 * 65], BF16, name="vG")
        vG4 = vG.rearrange("p (t h x) -> p t h x", t=TT_FULL, h=8)

        # ---- Phase A: LN1 (full seq) + QKV projections (own heads) ----
        with ExitStack() as cA:
            st_ps = cA.enter_context(
                tc.tile_pool(name="st_ps", bufs=1, space="PSUM"))
            mm_ps = cA.enter_context(
                tc.tile_pool(name="mm_ps", bufs=1, space="PSUM"))
            rowp = cA.enter_context(tc.tile_pool(name="rowp", bufs=1))
            bcp = cA.enter_context(tc.tile_pool(name="bcp", bufs=2))
            apl = cA.enter_context(tc.tile_pool(name="apl", bufs=1))
            hp = cA.enter_context(tc.tile_pool(name="hp", bufs=1))
            wp = cA.enter_context(tc.tile_pool(name="wp", bufs=1))
            lnp = cA.enter_context(tc.tile_pool(name="lnp", bufs=1))

            hTf = _alloc(hp, FT_C, [P, T], BF16, "hTf")

            _dmaq = [nc.sync, nc.scalar, nc.gpsimd]

            # x DMAs go out before any const/weight descriptors occupy the
            # queues so the first LN stats matmul isn't gated on queue drain.
            xf_pre = [[None] * KT_C for _ in range(4)]
            for nb in range(4):
                for kt in range(KT_C):
                    t = lnp.tile([P, 512], BF16, tag=f"x{kt}",
                                 name=f"xf{kt}_{nb}", bufs=3)
                    # block 0 rides only the two hardware DGE queues so the
                    # first LN stats matmuls aren't paced by the software
                    # (gpsimd) queue
                    q = ([nc.sync, nc.scalar][kt % 2] if nb == 0
                         else _dmaq[kt % 3])
                    q.dma_start(
                        out=t, in_=xT_b16[kt * P:(kt + 1) * P,
                                          nb * 512:(nb + 1) * 512])
                    xf_pre[nb][kt] = t

            # deferred const loads (needed from the LN apply onwards)
            nc.scalar.dma_start(out=ln1g_t,
                                in_=ln1_g.rearrange("(f p) -> p f", p=P))
            nc.scalar.dma_start(out=ln1b_t,
                                in_=ln1_b.rearrange("(f p) -> p f", p=P))
            nc.sync.dma_start(out=abq_t,
                              in_=attn_bs[0:HC].rearrange("(g p) -> p g", p=P))
            nc.sync.dma_start(out=abk_t,
                              in_=attn_bs[HC:2 * HC].rearrange("(g p) -> p g",
                                                               p=P))
            nc.scalar.dma_start(out=ln2g_t,
                                in_=ln2_g.rearrange("(f p) -> p f", p=P))
            nc.scalar.dma_start(out=ln2b_t,
                                in_=ln2_b.rearrange("(f p) -> p f", p=P))
            nc.scalar.dma_start(out=projb_t,
                                in_=proj_b.rearrange("(f p) -> p f", p=P))
            nc.scalar.dma_start(out=fc2b_t,
                                in_=fc2_b.rearrange("(f p) -> p f", p=P))
            nc.scalar.dma_start(out=fc1b_t,
                                in_=fc1_b.rearrange("(f p) -> p f", p=P))

            # streamed head-sliced weight tiles [P, 512]
            def wload(w_dram, idx, nm):
                w = wp.tile([P, HC], BF16, tag=f"w{idx}", name=nm, bufs=2)
                _dmaq[idx % 2].dma_start(
                    out=w, in_=w_dram[idx * P:(idx + 1) * P, :])
                return w

            wq = [wload(wq_s, kt, f"wq{kt}") for kt in range(KT_C)]

            # LN blocks interleaved with that block's Q projections: the PE
            # works on Q(nb) instead of stalling on the x DMA of block nb+1
            # (stats MMs for later blocks would otherwise head-block the
            # in-order queue while their x tiles stream in).
            for nb in range(4):
                _ln_b16(nc, lambda kt, _nb, _b=nb: xf_pre[_b][kt][:, :], 1,
                        hTf, nb * 512, ln1g_t, ln1b_t, eps_t, ones_b,
                        st_ps, rowp, bcp, apl, f"lf{nb}")
                qps = [mm_ps.tile([P, 512], F32, tag=f"mm{g}",
                                  name=f"qps{g}_{nb}") for g in range(NG)]
                for kt in range(KT_C):
                    for g in range(NG):
                        nc.tensor.matmul(
                            qps[g], wq[kt][:, g * P:(g + 1) * P],
                            hTf[kt][:, nb * 512:(nb + 1) * 512],
                            start=(kt == 0), stop=(kt == KT_C - 1))
                for g in range(NG):
                    nc.vector.tensor_scalar_add(
                        out=qT[g][:, nb * 512:(nb + 1) * 512], in0=qps[g],
                        scalar1=abq_t[:, g:g + 1])

            # K for all tokens, own heads
            wk = [wload(wk_s, kt, f"wk{kt}") for kt in range(KT_C)]
            for g in range(NG):
                kps = [mm_ps.tile([P, 512], F32, tag=f"mm{nb}",
                                  name=f"kps{g}_{nb}") for nb in range(4)]
                for kt in range(KT_C):
                    for nb in range(4):
                        nc.tensor.matmul(
                            kps[nb], wk[kt][:, g * P:(g + 1) * P],
                            hTf[kt][:, nb * 512:(nb + 1) * 512],
                            start=(kt == 0), stop=(kt == KT_C - 1))
                for nb in range(4):
                    nc.vector.tensor_scalar_add(
                        out=kT[g][:, nb * 512:(nb + 1) * 512], in0=kps[nb],
                        scalar1=abk_t[:, g:g + 1])

            # V rows (token-major), own heads
            nc.gpsimd.memset(vG4[:, :, :, 64:65], 1.0)  # softmax denom ones
            abv = attn_bs[2 * HC:3 * HC]
            nc.gpsimd.dma_start(
                out=bv_bc,
                in_=bass.AP(tensor=abv.tensor, offset=abv.offset,
                            ap=[[0, P]] + list(abv.ap[-1:])))
            wv = [wload(wv_s, kt, f"wv{kt}") for kt in range(KT_C)]
            for tt in range(TT_FULL):
                vps = mm_ps.tile([P, HC], F32, tag=f"mv{tt % 2}",
                                 name=f"vps{tt}")
                for kt in range(KT_C):
                    nc.tensor.matmul(
                        vps, hTf[kt][:, tt * P:(tt + 1) * P], wv[kt],
                        start=(kt == 0), stop=(kt == KT_C - 1))
                nc.vector.tensor_add(
                    out=vG4[:, tt, :, 0:64],
                    in0=vps.rearrange("p (h d) -> p h d", d=64),
                    in1=bv_bc.rearrange("p (h d) -> p h d", d=64))

        # ---- Phase B: attention (own heads, causal-tile-skipped) ----
        sM = ExitStack()
        attnp = sM.enter_context(tc.tile_pool(name="attnp", bufs=1, side="right"))
        attnT = _alloc(attnp, NG, [P, T], BF16, "attnT")
        pw = _alloc(attnp, KT_C, [P, C], BF16, "pw")
        xo32 = _alloc(attnp, FT_C, [P, TOWN], F32, "xo32")

        ccp = top.enter_context(tc.tile_pool(name="ccp", bufs=1, space="DRAM"))
        ccin = _alloc(ccp, NG, [4 * P, 1024], BF16, "ccin")
        ccout = _alloc(ccp, NG, [2 * P, 1024], BF16, "ccout")
        selt = const.tile([P, 2], F32, name="selt")
        nc.sync.dma_start(out=selt, in_=selbc[:, :])

        with ExitStack() as cB:
            mpool = cB.enter_context(tc.tile_pool(name="mpool", bufs=1))
            md = _alloc(mpool, 2, [P, 1024], BF16, "md")
            for k2 in range(2):
                nc.sync.dma_start(out=md[k2], in_=maskd[k2 * P:(k2 + 1) * P, :])
            for kt in range(KT_C):
                nc.sync.dma_start(out=pw[kt],
                                  in_=proj_wp[kt * P:(kt + 1) * P, :])
                nc.scalar.dma_start(out=xo32[kt],
                                    in_=xTo_f32[kt * P:(kt + 1) * P, :])

            sc_ps = cB.enter_context(
                tc.tile_pool(name="sc_ps", bufs=2, space="PSUM"))
            y_ps_pool = cB.enter_context(
                tc.tile_pool(name="y_ps_pool", bufs=1, space="PSUM"))
            ppool = cB.enter_context(tc.tile_pool(name="ppool", bufs=6))
            npool = cB.enter_context(tc.tile_pool(name="npool", bufs=2))
            ccsb = cB.enter_context(tc.tile_pool(name="ccsb", bufs=2))

            for g in range(NG):
                # two qc-pair passes: (0,1) over k2<4, (2,3) over k2<8
                for qa, qb in ((0, 1), (2, 3)):
                    y_ps = {
                        (qc, hh): y_ps_pool.tile(
                            [65, 512], F32, tag=f"y{qc % 2}{hh}",
                            name=f"y{g}_{qc}_{hh}")
                        for qc in (qa, qb) for hh in range(2)
                    }
                    for k2 in range(NK2[qb]):
                        for hh in range(2):
                            hsl = slice(64 * hh, 64 * (hh + 1))
                            scs = {}
                            if k2 < NK2[qa]:
                                scs[qa] = sc_ps.tile(
                                    [P, 1024], F32, tag="sc",
                                    name=f"sc{g}_{qa}_{k2}_{hh}")
                            scs[qb] = sc_ps.tile(
                                [P, 1024], F32, tag="sc",
                                name=f"sc{g}_{qb}_{k2}_{hh}")
                            for j in range(2):
                                kt = 2 * k2 + j
                                ksl = kT[g][hsl, kt * P:(kt + 1) * P]
                                for qc in scs:
                                    nc.tensor.matmul(
                                        scs[qc][:, j * 512:(j + 1) * 512],
                                        ksl,
                                        qT[g][hsl, qc * 512:(qc + 1) * 512],
                                        start=True, stop=True,
                                        tile_position=(64 * hh, 0))
                            pts = {}
                            for qc in scs:
                                pts[qc] = ppool.tile(
                                    [P, 1024], BF16, tag="pt",
                                    name=f"p{g}_{qc}_{k2}_{hh}")
                                nc.scalar.activation(out=pts[qc], in_=scs[qc],
                                                     func=Exp, scale=SCALE)
                                # mask diagonal tiles: qc's diagonal sits at
                                # k2 in {2qc, 2qc+1}
                                if k2 == 2 * qc:
                                    nc.vector.tensor_mul(
                                        out=pts[qc], in0=pts[qc], in1=md[0])
                                elif k2 == 2 * qc + 1:
                                    nc.vector.tensor_mul(
                                        out=pts[qc], in0=pts[qc], in1=md[1])
                            for j in range(2):
                                kt = 2 * k2 + j
                                vsl = vG4[:, kt, 2 * g + hh, :]
                                for qc in pts:
                                    nc.tensor.matmul(
                                        y_ps[(qc, hh)],
                                        vsl,
                                        pts[qc][:, j * 512:(j + 1) * 512],
                                        start=(kt == 0),
                                        stop=(kt == 2 * NK2[qc] - 1))
                    for qc in (qa, qb):
                        for hh in range(2):
                            dn = npool.tile([1, 512], F32, tag="dn",
                                            name=f"dn{g}_{qc}_{hh}")
                            nc.vector.tensor_copy(
                                out=dn, in_=y_ps[(qc, hh)][64:65, :])
                            r = npool.tile([1, 512], F32, tag="r",
                                           name=f"r{g}_{qc}_{hh}")
                            nc.vector.reciprocal_approx_fast(out=r, in_=dn)
                            rb = npool.tile([64, 512], F32, tag="rb",
                                            name=f"rb{g}_{qc}_{hh}")
                            nc.gpsimd.partition_broadcast(rb, r[0:1, :])
                            nc.vector.tensor_mul(
                                out=attnT[g][64 * hh:64 * (hh + 1),
                                             qc * 512:(qc + 1) * 512],
                                in0=y_ps[(qc, hh)][0:64, :], in1=rb)
                # Exchange this group's outputs with the pair core via
                # ReduceScatter-with-zero-slots: ccin rows are
                # [half h (2) x rank-slot s (2) x feat (128)]; each core
                # writes y into slot s==rank (selbc data picks the slot,
                # keeping the program SPMD-uniform) and zeros elsewhere.
                # RS(add) hands rank j rows [s*128+f] = rank s's heads for
                # token-half j - rank-stacked, uniform to read.
                ts = {}
                for s in range(2):
                    ts[s] = ccsb.tile([P, T], BF16, tag=f"ts{s}",
                                      name=f"ts{g}_{s}")
                    nc.vector.tensor_scalar_mul(
                        out=ts[s], in0=attnT[g], scalar1=selt[:, s:s + 1])
                for hhalf in range(2):
                    for s in range(2):
                        nc.sync.dma_start(
                            out=ccin[g][hhalf * 2 * P + s * P:
                                        hhalf * 2 * P + (s + 1) * P, :],
                            in_=ts[s][:, hhalf * 1024:(hhalf + 1) * 1024])
                nc.gpsimd.collective_compute(
                    "ReduceScatter", ADD, replica_groups=REPL,
                    ins=[ccin[g].opt()], outs=[ccout[g].opt()])

        s1.close()   # free qT/kT/vG

        # ---- gather exchanged attention rows; proj + residual -> x2T ----
        x2p = top.enter_context(tc.tile_pool(name="x2p", bufs=1))
        aT = _alloc(x2p, KT_C, [P, TOWN], BF16, "aT")
        x2T = _alloc(x2p, FT_C, [P, TOWN], F32R, "x2T")
        h2T = _alloc(x2p, FT_C, [P, TOWN], BF16, "h2T")
        for kt in range(KT_C):
            g, r = kt // 2, kt % 2
            _dmaq[kt % 3].dma_start(out=aT[kt],
                                    in_=ccout[g][r * P:(r + 1) * P, :])

        # prefetch the first half of fc1_w during proj/LN2
        w1p = top.enter_context(tc.tile_pool(name="w1p", bufs=1))

        def w1load(dh, kt):
            w = w1p.tile([P, DFF // 2], BF16, tag=f"w1_{kt}",
                         name=f"w1h{dh}_{kt}", bufs=1)
            nc.sync.dma_start(
                out=w,
                in_=fc1_w[kt * P:(kt + 1) * P,
                          dh * (DFF // 2):(dh + 1) * (DFF // 2)])
            return w

        w1h0 = [w1load(0, kt) for kt in range(KT_C)]

        with ExitStack() as cC:
            pj_ps = cC.enter_context(
                tc.tile_pool(name="pj_ps", bufs=2, space="PSUM"))
            st2 = cC.enter_context(
                tc.tile_pool(name="st2", bufs=2, space="PSUM"))
            rowp2 = cC.enter_context(tc.tile_pool(name="rowp2", bufs=1))
            bcp2 = cC.enter_context(tc.tile_pool(name="bcp2", bufs=2))
            apl2 = cC.enter_context(tc.tile_pool(name="apl2", bufs=1))

            # proj and LN2 interleaved per token-half: LN2(nb=0) overlaps
            # proj(nb=1), so fc1 only waits on the second half's apply.
            for nb in range(2):
                sl = slice(nb * 512, (nb + 1) * 512)
                for ft in range(FT_C):
                    pps = pj_ps.tile([P, 512], F32, tag=f"pj{ft % 2}",
                                     name=f"pps{ft}_{nb}")
                    for kt in range(KT_C):
                        nc.tensor.matmul(
                            pps, pw[kt][:, ft * P:(ft + 1) * P],
                            aT[kt][:, sl],
                            start=(kt == 0), stop=(kt == KT_C - 1))
                    nc.vector.scalar_tensor_tensor(
                        out=x2T[ft][:, sl], in0=pps,
                        scalar=projb_t[:, ft:ft + 1],
                        in1=xo32[ft][:, sl], op0=ADD, op1=ADD)

                # LN2 for this half, in f32 (x2T is the precise residual)
                ssum = st2.tile([1, 512], F32, tag="ss2", name=f"l2ss{nb}")
                ssq = st2.tile([1, 512], F32, tag="sq2", name=f"l2sq{nb}")
                for kt in range(KT_C):
                    nc.tensor.matmul(ssum, ones_r, x2T[kt][:, sl],
                                     start=(kt == 0), stop=(kt == KT_C - 1))
                for kt in range(KT_C):
                    sq = rowp2.tile([P, 512], F32R, tag="sqt",
                                    name=f"l2sqt{nb}_{kt}", bufs=3)
                    nc.vector.tensor_mul(out=sq, in0=x2T[kt][:, sl],
                                         in1=x2T[kt][:, sl])
                    nc.tensor.matmul(ssq, ones_r, sq,
                                     start=(kt == 0), stop=(kt == KT_C - 1))
                mu = rowp2.tile([1, 512], F32, tag="mu", name=f"l2mu{nb}")
                nc.scalar.mul(mu, ssum[0:1, :], 1.0 / C)
                msq = rowp2.tile([1, 512], F32, tag="msq", name=f"l2msq{nb}")
                nc.scalar.mul(msq, ssq[0:1, :], 1.0 / C)
                var = rowp2.tile([1, 512], F32, tag="var", name=f"l2var{nb}")
                nc.vector.tensor_mul(out=var, in0=mu, in1=mu)
                nc.vector.tensor_sub(out=var, in0=msq, in1=var)
                std = rowp2.tile([1, 512], F32, tag="std", name=f"l2std{nb}")
                nc.scalar.activation(out=std, in_=var, func=Sqrt,
                                     bias=eps_t[0:1, 0:1], scale=1.0)
                rs = rowp2.tile([1, 512], F32, tag="rs", name=f"l2rs{nb}")
                nc.vector.reciprocal_approx_fast(out=rs, in_=std)
                mu_b = bcp2.tile([P, 512], F32, tag="mub", name=f"l2mub{nb}")
                nc.gpsimd.partition_broadcast(mu_b, mu)
                rs_b = bcp2.tile([P, 512], F32, tag="rsb", name=f"l2rsb{nb}")
                nc.gpsimd.partition_broadcast(rs_b, rs)
                for ft in range(FT_C):
                    t = apl2.tile([P, 512], F32, tag="ap",
                                  name=f"l2ap{nb}_{ft}", bufs=3)
                    nc.vector.tensor_sub(out=t,
                                         in0=x2T[ft][:, sl].bitcast(F32),
                                         in1=mu_b)
                    nc.vector.tensor_mul(out=t, in0=t, in1=rs_b)
                    nc.scalar.activation(out=h2T[ft][:, sl], in_=t,
                                         func=Ident,
                                         bias=ln2b_t[:, ft:ft + 1],
                                         scale=ln2g_t[:, ft:ft + 1])
        sM.close()   # free attnT/pw/xo32

        # ---- Phase C: MLP ----
        with ExitStack() as cD:
            h1p = cD.enter_context(tc.tile_pool(name="h1p", bufs=1))
            h1 = _alloc(h1p, DFF // P, [P, TOWN], BF16, "h1")
            with ExitStack() as cD1:
                f1_ps = cD1.enter_context(
                    tc.tile_pool(name="f1_ps", bufs=2, space="PSUM"))
                for dh in range(2):
                    w1h = w1h0 if dh == 0 else [w1load(1, kt)
                                                for kt in range(KT_C)]
                    for dtl in range(16):
                        dt = dh * 16 + dtl
                        fps = [f1_ps.tile([P, 512], F32, tag=f"f1{nb}",
                                          name=f"fps{dt}_{nb}")
                               for nb in range(2)]
                        for kt in range(KT_C):
                            for nb in range(2):
                                nc.tensor.matmul(
                                    fps[nb], w1h[kt][:, dtl * P:(dtl + 1) * P],
                                    h2T[kt][:, nb * 512:(nb + 1) * 512],
                                    start=(kt == 0), stop=(kt == KT_C - 1))
                        for nb in range(2):
                            nc.scalar.activation(
                                out=h1[dt][:, nb * 512:(nb + 1) * 512],
                                in_=fps[nb], func=Relu,
                                bias=fc1b_t[:, dt:dt + 1], scale=1.0)

            with ExitStack() as cD2:
                f2_ps = cD2.enter_context(
                    tc.tile_pool(name="f2_ps", bufs=1, space="PSUM"))
                w2p = cD2.enter_context(tc.tile_pool(name="w2p", bufs=1))
                opool = cD2.enter_context(tc.tile_pool(name="opool", bufs=3))
                for fb, (f0, nf) in enumerate([(0, 4), (4, 2), (6, 2)]):
                    fp2 = [f2_ps.tile([P, TOWN], F32, tag=f"f2_{i}",
                                      name=f"fp2_{fb}_{i}")
                           for i in range(nf)]
                    for k8 in range(DFF // P):
                        w2t = w2p.tile([P, nf * P], BF16, tag=f"w2_{nf}",
                                       name=f"w2_{fb}_{k8}", bufs=6)
                        nc.sync.dma_start(
                            out=w2t, in_=fc2_w[k8 * P:(k8 + 1) * P,
                                               f0 * P:(f0 + nf) * P])
                        for i in range(nf):
                            for nb in range(2):
                                nc.tensor.matmul(
                                    fp2[i][:, nb * 512:(nb + 1) * 512],
                                    w2t[:, i * P:(i + 1) * P],
                                    h1[k8][:, nb * 512:(nb + 1) * 512],
                                    start=(k8 == 0), stop=(k8 == DFF // P - 1))
                    for i in range(nf):
                        ft = f0 + i
                        o = opool.tile([P, TOWN], F32, tag="o", name=f"o{ft}")
                        nc.vector.scalar_tensor_tensor(
                            out=o, in0=fp2[i], scalar=fc2b_t[:, ft:ft + 1],
                            in1=x2T[ft].bitcast(F32), op0=ADD, op1=ADD)
                        nc.sync.dma_start(out=out[ft * P:(ft + 1) * P, :],
                                          in_=o)

    nc.compile()
    return nc


_NC_CACHE = None


def _get_nc():
    global _NC_CACHE
    if _NC_CACHE is None:
        _NC_CACHE = build_nc()
    return _NC_CACHE


def _make_diag_mask():
    # [512 keys, 512 q] lower-triangular diag block, pair-packed to
    # [256, 1024]: row-block k2 (128 rows) holds [mask(kt=2k2) | mask(2k2+1)]
    k = np.arange(512, dtype=np.int64)[:, None]
    q = np.arange(512, dtype=np.int64)[None, :]
    m = (k <= q).astype(np.float32)
    return np.ascontiguousarray(
        m.reshape(2, 2, 128, 512).transpose(0, 2, 1, 3).reshape(256, 1024))


def _run(inputs, trace=False):
    nc = _get_nc()
    bf = ml_dtypes.bfloat16
    xs = {k: np.ascontiguousarray(np.asarray(v), dtype=np.float32)
          for k, v in inputs.items()}
    x = xs["x"]
    xT = {b: np.ascontiguousarray(x[b].T) for b in range(B)}
    md = _make_diag_mask().astype(bf)
    aw, ab = xs["attn_w"], xs["attn_b"]
    # proj rows permuted to the AllToAll rank-stacked feature order:
    # row (256g + 128r + d) <- global feature (8r + 2g)*64 + d
    perm = np.concatenate([
        np.arange((8 * r + 2 * g) * 64, (8 * r + 2 * g + 2) * 64)
        for g in range(NG) for r in range(2)])
    proj_wp = np.ascontiguousarray(xs["proj_w"][perm, :].astype(bf))
    w1 = np.ascontiguousarray(xs["fc1_w"].astype(bf))
    w2 = np.ascontiguousarray(xs["fc2_w"].astype(bf))
    in_maps = []
    for c in range(8):
        b, j = divmod(c, 2)
        hsl = slice(HC * j, HC * (j + 1))
        sel = np.zeros((P, 2), np.float32)
        sel[:, j] = 1.0
        in_maps.append({
            "xT_b16": np.ascontiguousarray(xT[b].astype(bf)),
            "xTo_f32": np.ascontiguousarray(xT[b][:, TOWN * j:TOWN * (j + 1)]),
            "maskd": md,
            "selbc": sel,
            "wq_s": np.ascontiguousarray(aw[:, 0:C][:, hsl].astype(bf)),
            "wk_s": np.ascontiguousarray(aw[:, C:2 * C][:, hsl].astype(bf)),
            "wv_s": np.ascontiguousarray(aw[:, 2 * C:3 * C][:, hsl].astype(bf)),
            "attn_bs": np.ascontiguousarray(np.concatenate(
                [ab[0:C][hsl], ab[C:2 * C][hsl], ab[2 * C:3 * C][hsl]])),
            "proj_wp": proj_wp, "proj_b": xs["proj_b"],
            "ln1_g": xs["ln1_g"], "ln1_b": xs["ln1_b"],
            "ln2_g": xs["ln2_g"], "ln2_b": xs["ln2_b"],
            "fc1_w": w1, "fc1_b": xs["fc1_b"],
            "fc2_w": w2, "fc2_b": xs["fc2_b"],
        })
    res = run_bass_kernel_spmd(nc, in_maps, list(range(8)), trace=trace)
    full = np.empty((B, T, C), dtype=np.float32)
    for c in range(8):
        b, j = divmod(c, 2)
        o = res.results[c]["out"]            # [C, TOWN] feature-major
        full[b, TOWN * j:TOWN * (j + 1)] = o.T
    return full, res.exec_time_ns


def kernel(**inputs):
    out, _ = _run(inputs, trace=False)
    return out
